# revision 32
# baseline (speedup 1.0000x reference)
"""DGCNN part-segmentation forward pass on 8 trn2 NeuronCores (Bass/Tile).

Sharding: data-parallel — sample b of the batch (B=8) runs on core b; each
core computes its full sample and the host stacks the 8 outputs.

Per-core pipeline (whole sample resident in SBUF):
  knn_t:  D = 2 x^T x - xx_i - xx_j built by PE as one augmented matmul
          (lhsT=[2x;1;-xx], rhs=[x;-xx;1]); the PSUM->SBUF move converts to
          fp16 written into the high halves of an iota-prefilled uint32
          buffer, so every value carries its column index in its low 11
          bits ("stuffed"); top-20 per row = forced self + top-19 via
          per-chunk max8 + match_replace merge on DVE; idx lists are
          DMA-shuffled (via a DRAM bounce) into the wrapped
          per-16-partition layout indirect_copy wants.
  EC_t:   EdgeConv via u/v decomposition: W[:, :C] x_j + (W[:,C:]-W[:,:C]) x_i,
          so only per-point matmuls + a gather of u columns; BN folded into
          weights host-side; channels packed 2 point-halves x 64ch = 128
          partitions; gather on GPSIMD, conv2 as block-diagonal 128x128
          matmul, k-max on DVE; processed in 2 point-sections to bound SBUF.
  head:   the 1024-ch global-max branch folds into a rank-1 bias column for
          W8 (the [1024, N] broadcast never exists); W8..W11 run per
          512-column stripe.
"""

import numpy as np

EPS = 1e-5
NEG = 0.2
B, C0, N = 8, 3, 2048
KNN = 20
NT = N // 128          # 16 row tiles per knn
HALF = N // 2
NSEC = 4               # EC point sections per half
SECP = HALF // NSEC    # 512 local points per section
SECW = SECP * KNN      # 10240 gathered elements per section
F32MIN = -3.0e38

_CACHE = {}


def _fold_bn(bn):
    s = bn['g'] / np.sqrt(bn['v'] + EPS)
    b = bn['b'] - bn['m'] * s
    return s.astype(np.float32), b.astype(np.float32)


def _prep_consts(params):
    """All host-side weight folding. Returns dict[str, np.ndarray]."""
    p = {k: (np.asarray(v, np.float32) if not isinstance(v, dict)
             else {kk: np.asarray(vv, np.float32) for kk, vv in v.items()})
         for k, v in params.items()}
    c = {}

    def ec_weights(W, bn, cin, scale_u=True):
        # u = (s*Wa) x_j (raw Wa for EC3), v = (s*(Wb-Wa)) x_i + b
        # the uv-rhs on device is [2x; ones] -> halve the x-part weights
        s, b = _fold_bn(bn)
        Wa, Wb = W[:, :cin], W[:, cin:]
        Wd = Wb - Wa
        uw = (Wa if not scale_u else s[:, None] * Wa) / 2.0
        vw = (s[:, None] * Wd) / 2.0
        u_lhsT = np.zeros((cin + 1, 128), np.float32)
        u_lhsT[:cin, 0:64] = uw.T
        u_lhsT[:cin, 64:128] = uw.T
        v_lhsT = np.zeros((cin + 1, 64), np.float32)
        v_lhsT[:cin, :] = vw.T
        v_lhsT[cin, :] = b
        return u_lhsT, v_lhsT, s

    def blockdiag(W, bn):
        s, b = _fold_bn(bn)
        Ws = (s[:, None] * W).astype(np.float32)
        bd = np.zeros((128, 128), np.float32)
        bd[0:64, 0:64] = Ws.T
        bd[64:128, 64:128] = Ws.T
        bdup = np.concatenate([b, b]).reshape(128, 1).astype(np.float32)
        return bd, bdup

    c['u1_lhsT'], c['v1_lhsT'], _ = ec_weights(p['W1'], p['bn1'], 3)
    c['W2bd'], c['b2dup'] = blockdiag(p['W2'], p['bn2'])
    c['u3_lhsT'], c['v3_lhsT'], _ = ec_weights(p['W3'], p['bn3'], 64)
    c['W4bd'], c['b4dup'] = blockdiag(p['W4'], p['bn4'])
    c['u5_lhsT'], c['v5_lhsT'], s5 = ec_weights(p['W5'], p['bn5'], 64,
                                                scale_u=False)
    c['s5dup'] = np.concatenate([s5, s5]).reshape(128, 1).astype(np.float32)

    s6, b6 = _fold_bn(p['bn6'])
    W6s = s6[:, None] * p['W6']                       # [1024, 192]
    c['W6l0'] = np.ascontiguousarray(W6s[:, 0:128].T)             # [128,1024]
    W6l1 = np.zeros((65, 1024), np.float32)
    W6l1[0:64] = W6s[:, 128:192].T
    c['W6l1'] = W6l1
    c['b6c'] = np.ascontiguousarray(b6.reshape(8, 128).T)         # [128, 8]

    s8, b8 = _fold_bn(p['bn8'])
    W8s = s8[:, None] * p['W8']                       # [256, 1216]
    W8g = W8s[:, 0:1024]
    c['W8l0'] = np.ascontiguousarray(W8s[:, 1024:1152].T)         # [128, 256]
    c['W8l1'] = np.ascontiguousarray(W8s[:, 1152:1216].T)         # [64, 256]
    w8g_rhs = np.zeros((128, 2048), np.float32)
    for m in range(8):
        w8g_rhs[:, 256 * m:256 * (m + 1)] = W8g[:, 128 * m:128 * (m + 1)].T
    c['w8g_rhs'] = w8g_rhs
    c['b8row'] = b8.reshape(1, 256).astype(np.float32)

    s9, b9 = _fold_bn(p['bn9'])
    W9s = s9[:, None] * p['W9']
    c['W9l0'] = np.ascontiguousarray(W9s[:, 0:128].T)
    c['W9l1'] = np.ascontiguousarray(W9s[:, 128:256].T)
    c['b9c'] = np.ascontiguousarray(b9.reshape(2, 128).T)         # [128, 2]

    s10, b10 = _fold_bn(p['bn10'])
    W10s = s10[:, None] * p['W10']
    c['W10l0'] = np.ascontiguousarray(W10s[:, 0:128].T)
    c['W10l1'] = np.ascontiguousarray(W10s[:, 128:256].T)
    c['b10c'] = b10.reshape(128, 1).astype(np.float32)

    c['W11l'] = np.ascontiguousarray(p['W11'].T)                  # [128, 50]
    c['ones64'] = np.ones((64, 1), np.float32)
    c['const1'] = np.ones((1, 1), np.float32)
    return c


def _prep_x(xb):
    """Per-sample augmented tensors. xb [3, N]."""
    xx = (xb * xb).sum(0, keepdims=True)
    x_dl = np.concatenate([2 * xb, np.ones((1, N), np.float32), -xx], 0)
    x_dr = np.concatenate([xb, -xx, np.ones((1, N), np.float32)], 0)
    return x_dl.astype(np.float32), x_dr.astype(np.float32)


# ---------------------------------------------------------------- wait fix
def _split_excess_waits(nc):
    """This walrus build encodes at most 2 sync-wait commands per compute
    instruction (and only 1 on CTRL-encoded ops like Drain/NoOp); Tile
    emits more on fan-in instructions. Move the excess onto same-engine
    NoOp carriers placed just before."""
    import concourse.mybir as mybir
    ctrl = (mybir.InstDrain, mybir.InstNoOp, mybir.InstEventSemaphore)
    n = 0
    for fn in nc.m.functions:
        for bb in fn.blocks:
            out = []
            for inst in bb.instructions:
                max_waits = 1
                si = inst.sync_info
                waits = list(si.on_wait) if si and si.on_wait else []
                if len(waits) > max_waits:
                    k = 0
                    while len(waits) > max_waits:
                        chunk, waits = waits[:max_waits], waits[max_waits:]
                        nop = mybir.InstNoOp(
                            name=f"{inst.name}-wsplit{k}", ins=[], outs=[])
                        nop.engine = inst.engine
                        nop.sync_info = mybir.SyncInfo(
                            on_wait=chunk, on_update=[])
                        out.append(nop)
                        k += 1
                    inst.sync_info = mybir.SyncInfo(
                        on_wait=waits, on_update=list(si.on_update or []))
                    n += 1
                out.append(inst)
            bb.instructions = out
    return n


# ---------------------------------------------------------------- program
def _build_program(const_shapes):
    import concourse.bass as bass
    import concourse.mybir as mybir
    import bass_rust as _br
    from concourse.tile import TileContext

    f32, f16, u32, u16 = (mybir.dt.float32, mybir.dt.float16,
                          mybir.dt.uint32, mybir.dt.uint16)
    AF = mybir.ActivationFunctionType
    ALU = mybir.AluOpType
    AX = mybir.AxisListType

    nc = bass.Bass(trn_type="TRN2")
    din = {}
    din['x_dl'] = nc.dram_tensor('x_dl', [5, N], f32, kind="ExternalInput")
    din['x_dr'] = nc.dram_tensor('x_dr', [5, N], f32, kind="ExternalInput")
    for name, shape in const_shapes.items():
        din[name] = nc.dram_tensor(name, list(shape), f32,
                                   kind="ExternalInput")
    out_d = nc.dram_tensor('out', [50, N], f32, kind="ExternalOutput")
    scr = [nc.dram_tensor(f'scr{i}', [2, HALF * KNN], u16) for i in range(3)]

    with TileContext(nc) as tc:
        with (tc.tile_pool(name="const", bufs=1) as cpool,
              tc.tile_pool(name="work", bufs=1) as wpool,
              tc.tile_pool(name="knnio", bufs=1) as kpool,
              tc.tile_pool(name="stage", bufs=2) as spool,
              tc.tile_pool(name="stripe", bufs=2) as stpool,
              tc.tile_pool(name="scr8", bufs=1) as scr8,
              tc.tile_pool(name="mm", bufs=2, space="PSUM") as mmp,
              tc.tile_pool(name="dmm", bufs=2, space="PSUM") as dmmp,
              tc.tile_pool(name="vq", bufs=1, space="PSUM") as vqp):

            # ---- constants in SBUF
            sb = {}
            for name in const_shapes:
                sh = list(const_shapes[name])
                sb[name] = cpool.tile(sh, f32, tag=name, name=name)
                nc.sync.dma_start(sb[name][:], din[name][:])
            # knn source tensors share the knnio slots across the 3 knns
            x_dl = kpool.tile([66, N], f32, tag="srcL")
            nc.sync.dma_start(x_dl[0:5, :], din['x_dl'][:])
            x_dr = kpool.tile([66, N], f32, tag="srcR")
            nc.sync.dma_start(x_dr[0:5, :], din['x_dr'][:])

            # ---- persistent work tiles
            stuf = [wpool.tile([128, N], u32, tag=f"stuf{i}", name=f"stuf{i}")
                    for i in (0, 1, 2, 3)]
            for st in stuf:
                nc.gpsimd.iota(st, pattern=[[1, N]], base=0,
                               channel_multiplier=0)
            cand = wpool.tile([128, 64], f32, tag="cand")
            m24 = wpool.tile([128, 24], f32, tag="m24")
            idxu = wpool.tile([128, 24], u32, tag="idxu")
            idx_all = wpool.tile([128, NT * KNN], u16, tag="idx_all")
            idxw = [wpool.tile([128, 1280], u16, tag=f"idxw{i}", name=f"idxw{i}")
                    for i in (0, 1)]
            u_dup = wpool.tile([128, N], f32, tag="u_dup")
            v_half = wpool.tile([128, HALF], f32, tag="v_half")
            e1g = wpool.tile([128, SECW], f32, tag="e1g")
            x_half = wpool.tile([128, HALF], f32, tag="x_half")
            headK0 = wpool.tile([128, N], f32, tag="headK0")
            headK1 = wpool.tile([65, N], f32, tag="headK1")
            gsb = wpool.tile([128, 8], f32, tag="gsb")
            ones_row = wpool.tile([1, N], f32, tag="ones_row")
            nc.vector.memset(ones_row[:], 1.0)
            xxr = wpool.tile([1, N], f32, tag="xxr")
            brow = wpool.tile([1, 256], f32, tag="brow")
            nc.sync.dma_start(headK1[64:65, :], ones_row[:])  # W8 rank-1 row

            # knn tile order: section 0 of both halves first, so its idx
            # shuffle + the downstream gather overlap the remaining tiles
            KNN_ORDER = [0, 1, 8, 9, 2, 3, 10, 11, 4, 5, 12, 13, 6, 7, 14, 15]
            DMA_ENGS = [nc.sync, nc.scalar]

            def shuffle_sec(widx, scratch, sec):
                """idx shuffle for point-section `sec` of both halves."""
                deps = []
                tps = 8 // NSEC                  # tiles per section-half
                for h in range(2):
                    # hop1: [p, (t, k)] -> DRAM linear i (within section)
                    t0 = 8 * h + tps * sec
                    d = DMA_ENGS[(2 * h + sec) % 2].dma_start(
                        scratch[h].rearrange("(s i) -> s i", s=NSEC)[sec]
                        .rearrange("(t p k) -> p t k", p=128, k=KNN),
                        idx_all[:, KNN * t0:KNN * (t0 + tps)]
                        .rearrange("p (t k) -> p t k", k=KNN),
                    )
                    deps.append(d)
                for g in range(8):
                    h = g // 4
                    h2 = DMA_ENGS[g % 2].dma_start(
                        widx[16 * g:16 * (g + 1),
                             (1280 // NSEC) * sec:
                             (1280 // NSEC) * (sec + 1)].rearrange(
                            "w (q u) -> w q u", u=KNN),
                        scratch[h].rearrange("(s q u w) -> s w q u", s=NSEC,
                                             w=16, u=KNN)[sec],
                    )
                    _br.add_dep_helper(h2.ins, deps[h].ins, True, "scr RAW")

            def knn(srcL, srcR, Kc, widx, scratch):
                """top-20 row neighbors of D; writes wrapped lists to widx."""
                # forced self idx (column 128 t + p) into slot 0 of every tile
                nc.gpsimd.iota(idx_all[:, 0::KNN], pattern=[[128, NT]],
                               base=0, channel_multiplier=1)
                for ti, t in enumerate(KNN_ORDER):
                    st = stuf[ti % 4]
                    s16 = st.bitcast(f16)
                    sf = st.bitcast(f32)
                    for c2 in range(2):
                        ps = dmmp.tile([128, 1024], f32, tag="dmm",
                                       name="psdmm")
                        for c in range(2):
                            nc.tensor.matmul(
                                ps[:, 512 * c:512 * (c + 1)],
                                srcL[0:Kc, 128 * t:128 * (t + 1)],
                                srcR[0:Kc, 1024 * c2 + 512 * c:
                                     1024 * c2 + 512 * (c + 1)],
                                start=True, stop=True)
                        nc.scalar.activation(
                            s16[:, 2048 * c2 + 1:2048 * (c2 + 1):2], ps,
                            AF.Copy)
                    for cc in range(8):
                        nc.vector.max(cand[:, 8 * cc:8 * (cc + 1)],
                                      sf[:, 256 * cc:256 * (cc + 1)])
                    nc.vector.max(m24[:, 0:8], cand)
                    nc.vector.match_replace(cand, m24[:, 0:8], cand, F32MIN)
                    nc.vector.max(m24[:, 8:16], cand)
                    nc.vector.match_replace(cand, m24[:, 8:16], cand, F32MIN)
                    nc.vector.max(m24[:, 16:24], cand)
                    # slot 0 is self (row max) -> keep slots 1..19
                    nc.vector.tensor_scalar(idxu[:, 0:20],
                                            m24.bitcast(u32)[:, 0:20],
                                            2047, None, op0=ALU.bitwise_and)
                    nc.vector.tensor_copy(
                        idx_all[:, KNN * t + 1:KNN * (t + 1)],
                        idxu[:, 1:20])
                    if ti % 4 == 3 and ti < 15:
                        shuffle_sec(widx, scratch, ti // 4)
                shuffle_sec(widx, scratch, NSEC - 1)

            def edgeconv(uL, vL, rhs_uv, Kc, widx, Wbd, bdup, out_half,
                         last=False, s5=None, add_eng=None, unpack_to=()):
                def unpack_sec(s):
                    xs_ = slice(SECP * s, SECP * (s + 1))
                    for dstf in unpack_to:
                        for h in range(2):
                            nc.sync.dma_start(
                                dstf(HALF * h + SECP * s),
                                out_half[64 * h:64 * (h + 1), xs_])
                """one EdgeConv; writes pooled output to out_half [128, HALF]."""
                for c in range(4):
                    ps = mmp.tile([128, 512], f32, tag="mm", name="psmm")
                    nc.tensor.matmul(ps, uL[0:Kc, :],
                                     rhs_uv[:, 512 * c:512 * (c + 1)],
                                     start=True, stop=True)
                    nc.scalar.copy(u_dup[:, 512 * c:512 * (c + 1)], ps)
                vps = vqp.tile([128, HALF], f32, tag="vq", name="vps")
                for h in range(2):
                    for c in range(2):
                        nc.tensor.matmul(
                            vps[64 * h:64 * (h + 1), 512 * c:512 * (c + 1)],
                            vL[0:Kc, :],
                            rhs_uv[:, HALF * h + 512 * c:
                                   HALF * h + 512 * (c + 1)],
                            start=True, stop=True)
                nc.scalar.copy(v_half[:], vps[:])
                for s in range(NSEC):
                    fs = slice(SECW * s, SECW * (s + 1))     # e1 free range
                    xs = slice(SECP * s, SECP * (s + 1))     # point range
                    # ISA caps indirect_copy around 1024 dst elements
                    gpos = 0
                    while gpos < SECW:
                        gw = min(1024, SECW - gpos)
                        nc.gpsimd.indirect_copy(
                            e1g[:, gpos:gpos + gw], u_dup,
                            widx[:, (1280 // NSEC) * s + gpos // 16:
                                 (1280 // NSEC) * s + (gpos + gw) // 16],
                            True)
                        gpos += gw
                    e3 = e1g.rearrange("p (n k) -> p n k", k=KNN)
                    if last:
                        # x3 = Prelu(s5 * max_k(u_j) + v')
                        nc.vector.tensor_reduce(x_half[:, xs], e3, AX.X,
                                                ALU.max)
                        nc.vector.scalar_tensor_tensor(
                            x_half[:, xs], x_half[:, xs], s5,
                            v_half[:, xs], op0=ALU.mult, op1=ALU.add)
                        nc.scalar.activation(x_half[:, xs], x_half[:, xs],
                                             AF.Prelu, alpha=NEG)
                        unpack_sec(s)
                        continue
                    # e1 = Prelu(u_j + v_i)  (BN + bias folded into u, v)
                    (add_eng or nc.gpsimd).tensor_add(
                        e3, e3,
                        v_half[:, xs, None].to_broadcast([128, SECP, KNN]))
                    nc.scalar.activation(e1g, e1g, AF.Prelu, alpha=NEG)
                    # conv2 + k-max in 500-wide chunks; k-max pools only
                    # over k, so BN bias + Prelu commute past it: reduce the
                    # raw PSUM on DVE, then one Prelu(+bias) per section
                    pos = 0
                    while pos < SECW:
                        w = min(500, SECW - pos)
                        npt = w // KNN
                        ps = mmp.tile([128, 512], f32, tag="mm", name="psmm")
                        nc.tensor.matmul(ps[:, 0:w], Wbd, e1g[:, pos:pos + w],
                                         start=True, stop=True)
                        p0 = SECP * s + pos // KNN
                        nc.vector.tensor_reduce(
                            x_half[:, p0:p0 + npt],
                            ps[:, 0:w].rearrange("p (n k) -> p n k", k=KNN),
                            AX.X, ALU.max)
                        pos += w
                    nc.scalar.activation(x_half[:, xs], x_half[:, xs],
                                         AF.Prelu, bias=bdup, alpha=NEG)
                    unpack_sec(s)

            def unpack(dst_rows, src_half):
                # [128 = ch x 2 halves, HALF] -> rows [64, N]
                for h in range(2):
                    nc.sync.dma_start(
                        dst_rows[:, HALF * h:HALF * (h + 1)],
                        src_half[64 * h:64 * (h + 1), :])

            def build_aug(lhsT2, rhs2, sq):
                """lhsT2 [66, N] = [2x;1;-xx], rhs2 [66, N] = [x;-xx;1];
                rhs2 rows 0:64 (= x, base partition 0) must already be
                filled by the unpack."""
                x_rows = rhs2[0:64, :]
                nc.sync.dma_start(lhsT2[64:65, :], ones_row[:])
                nc.sync.dma_start(rhs2[65:66, :], ones_row[:])
                nc.scalar.mul(lhsT2[0:64, :], x_rows, 2.0)
                nc.scalar.activation(sq, x_rows, AF.Square)
                for c in range(4):
                    ps = mmp.tile([128, 512], f32, tag="mm", name="psmm")
                    nc.tensor.matmul(ps[0:1, :], sb['ones64'],
                                     sq[:, 512 * c:512 * (c + 1)],
                                     start=True, stop=True)
                    nc.scalar.mul(xxr[:, 512 * c:512 * (c + 1)],
                                  ps[0:1, :], -1.0)
                nc.sync.dma_start(lhsT2[65:66, :], xxr[:])
                nc.sync.dma_start(rhs2[64:65, :], xxr[:])

            # ================= EC1
            knn(x_dl, x_dr, 5, idxw[0], scr[0])
            lhsT2 = kpool.tile([66, N], f32, tag="srcL")
            rhs2 = kpool.tile([66, N], f32, tag="srcR")
            sq = scr8.tile([64, N], f32, tag="scr8")
            edgeconv(sb['u1_lhsT'], sb['v1_lhsT'], x_dl[0:4], 4, idxw[0],
                     sb['W2bd'], sb['b2dup'], x_half, add_eng=nc.vector,
                     unpack_to=(
                         lambda c0: headK0[0:64, c0:c0 + SECP],
                         lambda c0: rhs2[0:64, c0:c0 + SECP]))
            build_aug(lhsT2, rhs2, sq)

            # ================= EC2
            knn(lhsT2, rhs2, 66, idxw[1], scr[1])
            lhsT2b = kpool.tile([66, N], f32, tag="srcL")
            rhs2b = kpool.tile([66, N], f32, tag="srcR")
            sqb = scr8.tile([64, N], f32, tag="scr8")
            edgeconv(sb['u3_lhsT'], sb['v3_lhsT'], lhsT2[0:65], 65, idxw[1],
                     sb['W4bd'], sb['b4dup'], x_half, add_eng=nc.vector,
                     unpack_to=(
                         lambda c0: headK0[64:128, c0:c0 + SECP],
                         lambda c0: rhs2b[0:64, c0:c0 + SECP]))
            build_aug(lhsT2b, rhs2b, sqb)

            # ================= EC3
            knn(lhsT2b, rhs2b, 66, idxw[0], scr[2])
            edgeconv(sb['u5_lhsT'], sb['v5_lhsT'], lhsT2b[0:65], 65, idxw[0],
                     None, None, x_half, last=True, s5=sb['s5dup'],
                     unpack_to=(
                         lambda c0: headK1[0:64, c0:c0 + SECP],))

            # ================= head: W6 + global max -> gsb
            # g = max_n Prelu(W6 cat + b6) = Prelu(max_n(W6 cat) + b6):
            # max the raw PSUM chunks on DVE, bias+Prelu only the [128, 8]
            g6c = wpool.tile([128, 32], f32, tag="g6c")
            for m in range(8):
                for c in range(4):
                    ps = mmp.tile([128, 512], f32, tag="mm", name="psmm")
                    nc.tensor.matmul(ps,
                                     sb['W6l0'][:, 128 * m:128 * (m + 1)],
                                     headK0[:, 512 * c:512 * (c + 1)],
                                     start=True, stop=False)
                    nc.tensor.matmul(ps,
                                     sb['W6l1'][:, 128 * m:128 * (m + 1)],
                                     headK1[:, 512 * c:512 * (c + 1)],
                                     start=False, stop=True)
                    nc.vector.tensor_reduce(g6c[:, 4 * m + c:4 * m + c + 1],
                                            ps, AX.X, ALU.max)
            nc.vector.tensor_reduce(gsb, g6c.rearrange("p (m c) -> p m c",
                                                       c=4), AX.X, ALU.max)
            nc.vector.tensor_add(gsb, gsb, sb['b6c'])
            nc.vector.scalar_tensor_tensor(gsb, gsb, NEG, gsb,
                                           op0=ALU.mult, op1=ALU.max)

            # bias row = (s8*W8g) g + b8   [1, 256]
            bps = mmp.tile([128, 512], f32, tag="mm", name="psmm")
            for m in range(8):
                nc.tensor.matmul(bps[0:1, 0:256], gsb[:, m:m + 1],
                                 sb['w8g_rhs'][:, 256 * m:256 * (m + 1)],
                                 start=(m == 0), stop=False)
            nc.tensor.matmul(bps[0:1, 0:256], sb['const1'], sb['b8row'],
                             start=False, stop=True)
            nc.scalar.copy(brow[:], bps[0:1, 0:256])

            # ================= head: W8 -> W9 -> W10 -> W11 per 512-stripe
            for c in range(4):
                cs = slice(512 * c, 512 * (c + 1))
                h8s = []
                for m in range(2):
                    ps = mmp.tile([128, 512], f32, tag="mm", name="psmm")
                    nc.tensor.matmul(ps, sb['W8l0'][:, 128 * m:128 * (m + 1)],
                                     headK0[:, cs], start=True, stop=False)
                    nc.tensor.matmul(ps, sb['W8l1'][:, 128 * m:128 * (m + 1)],
                                     headK1[0:64, cs], start=False,
                                     stop=False)
                    nc.tensor.matmul(ps, brow[0:1, 128 * m:128 * (m + 1)],
                                     ones_row[:, cs], start=False,
                                     stop=True)
                    hs = stpool.tile([128, 512], f32, tag="h8", name="h8s")
                    nc.scalar.activation(hs, ps, AF.Prelu, alpha=NEG)
                    h8s.append(hs)
                h9s = []
                for m in range(2):
                    ps = mmp.tile([128, 512], f32, tag="mm", name="psmm")
                    nc.tensor.matmul(ps, sb['W9l0'][:, 128 * m:128 * (m + 1)],
                                     h8s[0], start=True, stop=False)
                    nc.tensor.matmul(ps, sb['W9l1'][:, 128 * m:128 * (m + 1)],
                                     h8s[1], start=False, stop=True)
                    hs = stpool.tile([128, 512], f32, tag="h9", name="h9s")
                    nc.scalar.activation(hs, ps, AF.Prelu,
                                         bias=sb['b9c'][:, m:m + 1],
                                         alpha=NEG)
                    h9s.append(hs)
                ps = mmp.tile([128, 512], f32, tag="mm", name="psmm")
                nc.tensor.matmul(ps, sb['W10l0'], h9s[0], start=True,
                                 stop=False)
                nc.tensor.matmul(ps, sb['W10l1'], h9s[1], start=False,
                                 stop=True)
                h10s = stpool.tile([128, 512], f32, tag="h10", name="h10s")
                nc.scalar.activation(h10s, ps, AF.Prelu, bias=sb['b10c'],
                                     alpha=NEG)
                ps2 = mmp.tile([128, 512], f32, tag="mm")
                nc.tensor.matmul(ps2[0:50, :], sb['W11l'], h10s, start=True,
                                 stop=True)
                osb = stpool.tile([50, 512], f32, tag="osb", name="osb")
                nc.scalar.copy(osb[:], ps2[0:50, :])
                nc.sync.dma_start(out_d[:, cs], osb[:])

    return nc


def _get_program(const_shapes):
    if "prog" not in _CACHE:
        nc = _build_program(const_shapes)
        _split_excess_waits(nc)
        _CACHE["prog"] = nc
    return _CACHE["prog"]


def kernel(x, params):
    from concourse.bass_utils import run_bass_kernel_spmd

    x = np.asarray(x, np.float32)
    consts = _prep_consts(params)
    const_shapes = {k: v.shape for k, v in consts.items()}
    nc = _get_program(const_shapes)

    in_maps = []
    for b in range(B):
        x_dl, x_dr = _prep_x(x[b])
        m = dict(consts)
        m['x_dl'] = x_dl
        m['x_dr'] = x_dr
        in_maps.append(m)

    res = run_bass_kernel_spmd(nc, in_maps, core_ids=list(range(B)))
    return np.stack([res.results[b]['out'] for b in range(B)], 0)


# revision 33
# speedup vs baseline: 1.0059x; 1.0059x over previous
"""DGCNN part-segmentation forward pass on 8 trn2 NeuronCores (Bass/Tile).

Sharding: data-parallel — sample b of the batch (B=8) runs on core b; each
core computes its full sample and the host stacks the 8 outputs.

Per-core pipeline (whole sample resident in SBUF):
  knn_t:  D = 2 x^T x - xx_i - xx_j built by PE as one augmented matmul
          (lhsT=[2x;1;-xx], rhs=[x;-xx;1]); the PSUM->SBUF move converts to
          fp16 written into the high halves of an iota-prefilled uint32
          buffer, so every value carries its column index in its low 11
          bits ("stuffed"); top-20 per row = forced self + top-19 via
          per-chunk max8 + match_replace merge on DVE; idx lists are
          DMA-shuffled (via a DRAM bounce) into the wrapped
          per-16-partition layout indirect_copy wants.
  EC_t:   EdgeConv via u/v decomposition: W[:, :C] x_j + (W[:,C:]-W[:,:C]) x_i,
          so only per-point matmuls + a gather of u columns; BN folded into
          weights host-side; channels packed 2 point-halves x 64ch = 128
          partitions; gather on GPSIMD, conv2 as block-diagonal 128x128
          matmul, k-max on DVE; processed in 2 point-sections to bound SBUF.
  head:   the 1024-ch global-max branch folds into a rank-1 bias column for
          W8 (the [1024, N] broadcast never exists); W8..W11 run per
          512-column stripe.
"""

import numpy as np

EPS = 1e-5
NEG = 0.2
B, C0, N = 8, 3, 2048
KNN = 20
NT = N // 128          # 16 row tiles per knn
HALF = N // 2
NSEC = 4               # EC point sections per half
SECP = HALF // NSEC    # 512 local points per section
SECW = SECP * KNN      # 10240 gathered elements per section
F32MIN = -3.0e38

_CACHE = {}


def _fold_bn(bn):
    s = bn['g'] / np.sqrt(bn['v'] + EPS)
    b = bn['b'] - bn['m'] * s
    return s.astype(np.float32), b.astype(np.float32)


def _prep_consts(params):
    """All host-side weight folding. Returns dict[str, np.ndarray]."""
    p = {k: (np.asarray(v, np.float32) if not isinstance(v, dict)
             else {kk: np.asarray(vv, np.float32) for kk, vv in v.items()})
         for k, v in params.items()}
    c = {}

    def ec_weights(W, bn, cin, scale_u=True):
        # u = (s*Wa) x_j (raw Wa for EC3), v = (s*(Wb-Wa)) x_i + b
        # the uv-rhs on device is [2x; ones] -> halve the x-part weights
        s, b = _fold_bn(bn)
        Wa, Wb = W[:, :cin], W[:, cin:]
        Wd = Wb - Wa
        uw = (Wa if not scale_u else s[:, None] * Wa) / 2.0
        vw = (s[:, None] * Wd) / 2.0
        u_lhsT = np.zeros((cin + 1, 128), np.float32)
        u_lhsT[:cin, 0:64] = uw.T
        u_lhsT[:cin, 64:128] = uw.T
        v_lhsT = np.zeros((cin + 1, 64), np.float32)
        v_lhsT[:cin, :] = vw.T
        v_lhsT[cin, :] = b
        return u_lhsT, v_lhsT, s

    def blockdiag(W, bn):
        s, b = _fold_bn(bn)
        Ws = (s[:, None] * W).astype(np.float32)
        bd = np.zeros((128, 128), np.float32)
        bd[0:64, 0:64] = Ws.T
        bd[64:128, 64:128] = Ws.T
        bdup = np.concatenate([b, b]).reshape(128, 1).astype(np.float32)
        return bd, bdup

    c['u1_lhsT'], c['v1_lhsT'], _ = ec_weights(p['W1'], p['bn1'], 3)
    c['W2bd'], c['b2dup'] = blockdiag(p['W2'], p['bn2'])
    c['u3_lhsT'], c['v3_lhsT'], _ = ec_weights(p['W3'], p['bn3'], 64)
    c['W4bd'], c['b4dup'] = blockdiag(p['W4'], p['bn4'])
    c['u5_lhsT'], c['v5_lhsT'], s5 = ec_weights(p['W5'], p['bn5'], 64,
                                                scale_u=False)
    c['s5dup'] = np.concatenate([s5, s5]).reshape(128, 1).astype(np.float32)

    s6, b6 = _fold_bn(p['bn6'])
    W6s = s6[:, None] * p['W6']                       # [1024, 192]
    c['W6l0'] = np.ascontiguousarray(W6s[:, 0:128].T)             # [128,1024]
    W6l1 = np.zeros((65, 1024), np.float32)
    W6l1[0:64] = W6s[:, 128:192].T
    c['W6l1'] = W6l1
    c['b6c'] = np.ascontiguousarray(b6.reshape(8, 128).T)         # [128, 8]

    s8, b8 = _fold_bn(p['bn8'])
    W8s = s8[:, None] * p['W8']                       # [256, 1216]
    W8g = W8s[:, 0:1024]
    c['W8l0'] = np.ascontiguousarray(W8s[:, 1024:1152].T)         # [128, 256]
    c['W8l1'] = np.ascontiguousarray(W8s[:, 1152:1216].T)         # [64, 256]
    w8g_rhs = np.zeros((128, 2048), np.float32)
    for m in range(8):
        w8g_rhs[:, 256 * m:256 * (m + 1)] = W8g[:, 128 * m:128 * (m + 1)].T
    c['w8g_rhs'] = w8g_rhs
    c['b8row'] = b8.reshape(1, 256).astype(np.float32)

    s9, b9 = _fold_bn(p['bn9'])
    W9s = s9[:, None] * p['W9']
    c['W9l0'] = np.ascontiguousarray(W9s[:, 0:128].T)
    c['W9l1'] = np.ascontiguousarray(W9s[:, 128:256].T)
    c['b9c'] = np.ascontiguousarray(b9.reshape(2, 128).T)         # [128, 2]

    s10, b10 = _fold_bn(p['bn10'])
    W10s = s10[:, None] * p['W10']
    c['W10l0'] = np.ascontiguousarray(W10s[:, 0:128].T)
    c['W10l1'] = np.ascontiguousarray(W10s[:, 128:256].T)
    c['b10c'] = b10.reshape(128, 1).astype(np.float32)

    c['W11l'] = np.ascontiguousarray(p['W11'].T)                  # [128, 50]
    c['ones64'] = np.ones((64, 1), np.float32)
    c['const1'] = np.ones((1, 1), np.float32)
    return c


def _prep_x(xb):
    """Per-sample augmented tensors. xb [3, N]."""
    xx = (xb * xb).sum(0, keepdims=True)
    x_dl = np.concatenate([2 * xb, np.ones((1, N), np.float32), -xx], 0)
    x_dr = np.concatenate([xb, -xx, np.ones((1, N), np.float32)], 0)
    return x_dl.astype(np.float32), x_dr.astype(np.float32)


# ---------------------------------------------------------------- wait fix
def _split_excess_waits(nc):
    """This walrus build encodes at most 2 sync-wait commands per compute
    instruction (and only 1 on CTRL-encoded ops like Drain/NoOp); Tile
    emits more on fan-in instructions. Move the excess onto same-engine
    NoOp carriers placed just before."""
    import concourse.mybir as mybir
    ctrl = (mybir.InstDrain, mybir.InstNoOp, mybir.InstEventSemaphore)
    n = 0
    for fn in nc.m.functions:
        for bb in fn.blocks:
            out = []
            for inst in bb.instructions:
                max_waits = 1
                si = inst.sync_info
                waits = list(si.on_wait) if si and si.on_wait else []
                if len(waits) > max_waits:
                    k = 0
                    while len(waits) > max_waits:
                        chunk, waits = waits[:max_waits], waits[max_waits:]
                        nop = mybir.InstNoOp(
                            name=f"{inst.name}-wsplit{k}", ins=[], outs=[])
                        nop.engine = inst.engine
                        nop.sync_info = mybir.SyncInfo(
                            on_wait=chunk, on_update=[])
                        out.append(nop)
                        k += 1
                    inst.sync_info = mybir.SyncInfo(
                        on_wait=waits, on_update=list(si.on_update or []))
                    n += 1
                out.append(inst)
            bb.instructions = out
    return n


# ---------------------------------------------------------------- program
def _build_program(const_shapes):
    import concourse.bass as bass
    import concourse.mybir as mybir
    import bass_rust as _br
    from concourse.tile import TileContext

    f32, f16, u32, u16 = (mybir.dt.float32, mybir.dt.float16,
                          mybir.dt.uint32, mybir.dt.uint16)
    AF = mybir.ActivationFunctionType
    ALU = mybir.AluOpType
    AX = mybir.AxisListType

    nc = bass.Bass(trn_type="TRN2")
    din = {}
    din['x_dl'] = nc.dram_tensor('x_dl', [5, N], f32, kind="ExternalInput")
    din['x_dr'] = nc.dram_tensor('x_dr', [5, N], f32, kind="ExternalInput")
    for name, shape in const_shapes.items():
        din[name] = nc.dram_tensor(name, list(shape), f32,
                                   kind="ExternalInput")
    out_d = nc.dram_tensor('out', [50, N], f32, kind="ExternalOutput")
    scr = [nc.dram_tensor(f'scr{i}', [2, HALF * KNN], u16) for i in range(3)]

    with TileContext(nc) as tc:
        with (tc.tile_pool(name="const", bufs=1) as cpool,
              tc.tile_pool(name="work", bufs=1) as wpool,
              tc.tile_pool(name="knnio", bufs=1) as kpool,
              tc.tile_pool(name="stage", bufs=2) as spool,
              tc.tile_pool(name="stripe", bufs=2) as stpool,
              tc.tile_pool(name="scr8", bufs=1) as scr8,
              tc.tile_pool(name="mm", bufs=2, space="PSUM") as mmp,
              tc.tile_pool(name="dmm", bufs=2, space="PSUM") as dmmp,
              tc.tile_pool(name="vq", bufs=1, space="PSUM") as vqp):

            # ---- constants in SBUF
            sb = {}
            for name in const_shapes:
                sh = list(const_shapes[name])
                sb[name] = cpool.tile(sh, f32, tag=name, name=name)
                nc.sync.dma_start(sb[name][:], din[name][:])
            # knn source tensors share the knnio slots across the 3 knns
            x_dl = kpool.tile([66, N], f32, tag="srcL")
            nc.sync.dma_start(x_dl[0:5, :], din['x_dl'][:])
            x_dr = kpool.tile([66, N], f32, tag="srcR")
            nc.sync.dma_start(x_dr[0:5, :], din['x_dr'][:])

            # ---- persistent work tiles
            stuf = [wpool.tile([128, N], u32, tag=f"stuf{i}", name=f"stuf{i}")
                    for i in (0, 1, 2, 3)]
            for st in stuf:
                nc.gpsimd.iota(st, pattern=[[1, N]], base=0,
                               channel_multiplier=0)
            cand = wpool.tile([128, 64], f32, tag="cand")
            m24 = wpool.tile([128, 24], f32, tag="m24")
            idxu = wpool.tile([128, 24], u32, tag="idxu")
            idx_all = wpool.tile([128, NT * KNN], u16, tag="idx_all")
            idxw = [wpool.tile([128, 1280], u16, tag=f"idxw{i}", name=f"idxw{i}")
                    for i in (0, 1)]
            u_dup = wpool.tile([128, N], f32, tag="u_dup")
            v_half = wpool.tile([128, HALF], f32, tag="v_half")
            e1g = wpool.tile([128, SECW], f32, tag="e1g")
            x_half = wpool.tile([128, HALF], f32, tag="x_half")
            headK0 = wpool.tile([128, N], f32, tag="headK0")
            headK1 = wpool.tile([65, N], f32, tag="headK1")
            gsb = wpool.tile([128, 8], f32, tag="gsb")
            ones_row = wpool.tile([1, N], f32, tag="ones_row")
            nc.vector.memset(ones_row[:], 1.0)
            xxr = wpool.tile([1, N], f32, tag="xxr")
            brow = wpool.tile([1, 256], f32, tag="brow")
            nc.sync.dma_start(headK1[64:65, :], ones_row[:])  # W8 rank-1 row

            # knn tile order: section 0 of both halves first, so its idx
            # shuffle + the downstream gather overlap the remaining tiles
            KNN_ORDER = [0, 1, 8, 9, 2, 3, 10, 11, 4, 5, 12, 13, 6, 7, 14, 15]
            DMA_ENGS = [nc.sync, nc.scalar]

            def shuffle_sec(widx, scratch, sec):
                """idx shuffle for point-section `sec` of both halves."""
                deps = []
                tps = 8 // NSEC                  # tiles per section-half
                for h in range(2):
                    # hop1, one DMA per tile so only the last tile's small
                    # transfer sits on the critical chain
                    t0 = 8 * h + tps * sec
                    dsec = scratch[h].rearrange("(s i) -> s i", s=NSEC)[sec]                         .rearrange("(t p k) -> t p k", p=128, k=KNN)
                    dh = []
                    for tt in range(tps):
                        dh.append(DMA_ENGS[(2 * h + sec + tt) % 2].dma_start(
                            dsec[tt],
                            idx_all[:, KNN * (t0 + tt):KNN * (t0 + tt + 1)]
                            .rearrange("p k -> p k"),
                        ))
                    deps.append(dh)
                for g in range(8):
                    h = g // 4
                    h2 = DMA_ENGS[g % 2].dma_start(
                        widx[16 * g:16 * (g + 1),
                             (1280 // NSEC) * sec:
                             (1280 // NSEC) * (sec + 1)].rearrange(
                            "w (q u) -> w q u", u=KNN),
                        scratch[h].rearrange("(s q u w) -> s w q u", s=NSEC,
                                             w=16, u=KNN)[sec],
                    )
                    for d_ in deps[h]:
                        _br.add_dep_helper(h2.ins, d_.ins, True, "scr RAW")

            def knn(srcL, srcR, Kc, widx, scratch):
                """top-20 row neighbors of D; writes wrapped lists to widx."""
                # forced self idx (column 128 t + p) into slot 0 of every tile
                nc.gpsimd.iota(idx_all[:, 0::KNN], pattern=[[128, NT]],
                               base=0, channel_multiplier=1)
                for ti, t in enumerate(KNN_ORDER):
                    st = stuf[ti % 4]
                    s16 = st.bitcast(f16)
                    sf = st.bitcast(f32)
                    for c2 in range(2):
                        ps = dmmp.tile([128, 1024], f32, tag="dmm",
                                       name="psdmm")
                        for c in range(2):
                            nc.tensor.matmul(
                                ps[:, 512 * c:512 * (c + 1)],
                                srcL[0:Kc, 128 * t:128 * (t + 1)],
                                srcR[0:Kc, 1024 * c2 + 512 * c:
                                     1024 * c2 + 512 * (c + 1)],
                                start=True, stop=True)
                        nc.scalar.activation(
                            s16[:, 2048 * c2 + 1:2048 * (c2 + 1):2], ps,
                            AF.Copy)
                    for cc in range(8):
                        nc.vector.max(cand[:, 8 * cc:8 * (cc + 1)],
                                      sf[:, 256 * cc:256 * (cc + 1)])
                    nc.vector.max(m24[:, 0:8], cand)
                    nc.vector.match_replace(cand, m24[:, 0:8], cand, F32MIN)
                    nc.vector.max(m24[:, 8:16], cand)
                    nc.vector.match_replace(cand, m24[:, 8:16], cand, F32MIN)
                    nc.vector.max(m24[:, 16:24], cand)
                    # slot 0 is self (row max) -> keep slots 1..19
                    nc.vector.tensor_scalar(idxu[:, 0:20],
                                            m24.bitcast(u32)[:, 0:20],
                                            2047, None, op0=ALU.bitwise_and)
                    nc.vector.tensor_copy(
                        idx_all[:, KNN * t + 1:KNN * (t + 1)],
                        idxu[:, 1:20])
                    if ti % 4 == 3 and ti < 15:
                        shuffle_sec(widx, scratch, ti // 4)
                shuffle_sec(widx, scratch, NSEC - 1)

            def edgeconv(uL, vL, rhs_uv, Kc, widx, Wbd, bdup, out_half,
                         last=False, s5=None, add_eng=None, unpack_to=()):
                def unpack_sec(s):
                    xs_ = slice(SECP * s, SECP * (s + 1))
                    for dstf in unpack_to:
                        for h in range(2):
                            nc.sync.dma_start(
                                dstf(HALF * h + SECP * s),
                                out_half[64 * h:64 * (h + 1), xs_])
                """one EdgeConv; writes pooled output to out_half [128, HALF]."""
                for c in range(4):
                    ps = mmp.tile([128, 512], f32, tag="mm", name="psmm")
                    nc.tensor.matmul(ps, uL[0:Kc, :],
                                     rhs_uv[:, 512 * c:512 * (c + 1)],
                                     start=True, stop=True)
                    nc.scalar.copy(u_dup[:, 512 * c:512 * (c + 1)], ps)
                vps = vqp.tile([128, HALF], f32, tag="vq", name="vps")
                for h in range(2):
                    for c in range(2):
                        nc.tensor.matmul(
                            vps[64 * h:64 * (h + 1), 512 * c:512 * (c + 1)],
                            vL[0:Kc, :],
                            rhs_uv[:, HALF * h + 512 * c:
                                   HALF * h + 512 * (c + 1)],
                            start=True, stop=True)
                nc.scalar.copy(v_half[:], vps[:])
                for s in range(NSEC):
                    fs = slice(SECW * s, SECW * (s + 1))     # e1 free range
                    xs = slice(SECP * s, SECP * (s + 1))     # point range
                    # ISA caps indirect_copy around 1024 dst elements
                    gpos = 0
                    while gpos < SECW:
                        gw = min(1024, SECW - gpos)
                        nc.gpsimd.indirect_copy(
                            e1g[:, gpos:gpos + gw], u_dup,
                            widx[:, (1280 // NSEC) * s + gpos // 16:
                                 (1280 // NSEC) * s + (gpos + gw) // 16],
                            True)
                        gpos += gw
                    e3 = e1g.rearrange("p (n k) -> p n k", k=KNN)
                    if last:
                        # x3 = Prelu(s5 * max_k(u_j) + v')
                        nc.vector.tensor_reduce(x_half[:, xs], e3, AX.X,
                                                ALU.max)
                        nc.vector.scalar_tensor_tensor(
                            x_half[:, xs], x_half[:, xs], s5,
                            v_half[:, xs], op0=ALU.mult, op1=ALU.add)
                        nc.scalar.activation(x_half[:, xs], x_half[:, xs],
                                             AF.Prelu, alpha=NEG)
                        unpack_sec(s)
                        continue
                    # e1 = Prelu(u_j + v_i)  (BN + bias folded into u, v)
                    (add_eng or nc.gpsimd).tensor_add(
                        e3, e3,
                        v_half[:, xs, None].to_broadcast([128, SECP, KNN]))
                    nc.scalar.activation(e1g, e1g, AF.Prelu, alpha=NEG)
                    # conv2 + k-max in 500-wide chunks; k-max pools only
                    # over k, so BN bias + Prelu commute past it: reduce the
                    # raw PSUM on DVE, then one Prelu(+bias) per section
                    pos = 0
                    while pos < SECW:
                        w = min(500, SECW - pos)
                        npt = w // KNN
                        ps = mmp.tile([128, 512], f32, tag="mm", name="psmm")
                        nc.tensor.matmul(ps[:, 0:w], Wbd, e1g[:, pos:pos + w],
                                         start=True, stop=True)
                        p0 = SECP * s + pos // KNN
                        nc.vector.tensor_reduce(
                            x_half[:, p0:p0 + npt],
                            ps[:, 0:w].rearrange("p (n k) -> p n k", k=KNN),
                            AX.X, ALU.max)
                        pos += w
                    nc.scalar.activation(x_half[:, xs], x_half[:, xs],
                                         AF.Prelu, bias=bdup, alpha=NEG)
                    unpack_sec(s)

            def unpack(dst_rows, src_half):
                # [128 = ch x 2 halves, HALF] -> rows [64, N]
                for h in range(2):
                    nc.sync.dma_start(
                        dst_rows[:, HALF * h:HALF * (h + 1)],
                        src_half[64 * h:64 * (h + 1), :])

            def build_aug(lhsT2, rhs2, sq):
                """lhsT2 [66, N] = [2x;1;-xx], rhs2 [66, N] = [x;-xx;1];
                rhs2 rows 0:64 (= x, base partition 0) must already be
                filled by the unpack."""
                x_rows = rhs2[0:64, :]
                nc.sync.dma_start(lhsT2[64:65, :], ones_row[:])
                nc.sync.dma_start(rhs2[65:66, :], ones_row[:])
                nc.scalar.mul(lhsT2[0:64, :], x_rows, 2.0)
                nc.scalar.activation(sq, x_rows, AF.Square)
                for c in range(4):
                    ps = mmp.tile([128, 512], f32, tag="mm", name="psmm")
                    nc.tensor.matmul(ps[0:1, :], sb['ones64'],
                                     sq[:, 512 * c:512 * (c + 1)],
                                     start=True, stop=True)
                    nc.scalar.mul(xxr[:, 512 * c:512 * (c + 1)],
                                  ps[0:1, :], -1.0)
                nc.sync.dma_start(lhsT2[65:66, :], xxr[:])
                nc.sync.dma_start(rhs2[64:65, :], xxr[:])

            # ================= EC1
            knn(x_dl, x_dr, 5, idxw[0], scr[0])
            lhsT2 = kpool.tile([66, N], f32, tag="srcL")
            rhs2 = kpool.tile([66, N], f32, tag="srcR")
            sq = scr8.tile([64, N], f32, tag="scr8")
            edgeconv(sb['u1_lhsT'], sb['v1_lhsT'], x_dl[0:4], 4, idxw[0],
                     sb['W2bd'], sb['b2dup'], x_half, add_eng=nc.vector,
                     unpack_to=(
                         lambda c0: headK0[0:64, c0:c0 + SECP],
                         lambda c0: rhs2[0:64, c0:c0 + SECP]))
            build_aug(lhsT2, rhs2, sq)

            # ================= EC2
            knn(lhsT2, rhs2, 66, idxw[1], scr[1])
            lhsT2b = kpool.tile([66, N], f32, tag="srcL")
            rhs2b = kpool.tile([66, N], f32, tag="srcR")
            sqb = scr8.tile([64, N], f32, tag="scr8")
            edgeconv(sb['u3_lhsT'], sb['v3_lhsT'], lhsT2[0:65], 65, idxw[1],
                     sb['W4bd'], sb['b4dup'], x_half, add_eng=nc.vector,
                     unpack_to=(
                         lambda c0: headK0[64:128, c0:c0 + SECP],
                         lambda c0: rhs2b[0:64, c0:c0 + SECP]))
            build_aug(lhsT2b, rhs2b, sqb)

            # ================= EC3
            knn(lhsT2b, rhs2b, 66, idxw[0], scr[2])
            edgeconv(sb['u5_lhsT'], sb['v5_lhsT'], lhsT2b[0:65], 65, idxw[0],
                     None, None, x_half, last=True, s5=sb['s5dup'],
                     unpack_to=(
                         lambda c0: headK1[0:64, c0:c0 + SECP],))

            # ================= head: W6 + global max -> gsb
            # g = max_n Prelu(W6 cat + b6) = Prelu(max_n(W6 cat) + b6):
            # max the raw PSUM chunks on DVE, bias+Prelu only the [128, 8]
            g6c = wpool.tile([128, 32], f32, tag="g6c")
            for m in range(8):
                for c in range(4):
                    ps = mmp.tile([128, 512], f32, tag="mm", name="psmm")
                    nc.tensor.matmul(ps,
                                     sb['W6l0'][:, 128 * m:128 * (m + 1)],
                                     headK0[:, 512 * c:512 * (c + 1)],
                                     start=True, stop=False)
                    nc.tensor.matmul(ps,
                                     sb['W6l1'][:, 128 * m:128 * (m + 1)],
                                     headK1[:, 512 * c:512 * (c + 1)],
                                     start=False, stop=True)
                    nc.vector.tensor_reduce(g6c[:, 4 * m + c:4 * m + c + 1],
                                            ps, AX.X, ALU.max)
            nc.vector.tensor_reduce(gsb, g6c.rearrange("p (m c) -> p m c",
                                                       c=4), AX.X, ALU.max)
            nc.vector.tensor_add(gsb, gsb, sb['b6c'])
            nc.vector.scalar_tensor_tensor(gsb, gsb, NEG, gsb,
                                           op0=ALU.mult, op1=ALU.max)

            # bias row = (s8*W8g) g + b8   [1, 256]
            bps = mmp.tile([128, 512], f32, tag="mm", name="psmm")
            for m in range(8):
                nc.tensor.matmul(bps[0:1, 0:256], gsb[:, m:m + 1],
                                 sb['w8g_rhs'][:, 256 * m:256 * (m + 1)],
                                 start=(m == 0), stop=False)
            nc.tensor.matmul(bps[0:1, 0:256], sb['const1'], sb['b8row'],
                             start=False, stop=True)
            nc.scalar.copy(brow[:], bps[0:1, 0:256])

            # ================= head: W8 -> W9 -> W10 -> W11 per 512-stripe
            for c in range(4):
                cs = slice(512 * c, 512 * (c + 1))
                h8s = []
                for m in range(2):
                    ps = mmp.tile([128, 512], f32, tag="mm", name="psmm")
                    nc.tensor.matmul(ps, sb['W8l0'][:, 128 * m:128 * (m + 1)],
                                     headK0[:, cs], start=True, stop=False)
                    nc.tensor.matmul(ps, sb['W8l1'][:, 128 * m:128 * (m + 1)],
                                     headK1[0:64, cs], start=False,
                                     stop=False)
                    nc.tensor.matmul(ps, brow[0:1, 128 * m:128 * (m + 1)],
                                     ones_row[:, cs], start=False,
                                     stop=True)
                    hs = stpool.tile([128, 512], f32, tag="h8", name="h8s")
                    nc.scalar.activation(hs, ps, AF.Prelu, alpha=NEG)
                    h8s.append(hs)
                h9s = []
                for m in range(2):
                    ps = mmp.tile([128, 512], f32, tag="mm", name="psmm")
                    nc.tensor.matmul(ps, sb['W9l0'][:, 128 * m:128 * (m + 1)],
                                     h8s[0], start=True, stop=False)
                    nc.tensor.matmul(ps, sb['W9l1'][:, 128 * m:128 * (m + 1)],
                                     h8s[1], start=False, stop=True)
                    hs = stpool.tile([128, 512], f32, tag="h9", name="h9s")
                    nc.scalar.activation(hs, ps, AF.Prelu,
                                         bias=sb['b9c'][:, m:m + 1],
                                         alpha=NEG)
                    h9s.append(hs)
                ps = mmp.tile([128, 512], f32, tag="mm", name="psmm")
                nc.tensor.matmul(ps, sb['W10l0'], h9s[0], start=True,
                                 stop=False)
                nc.tensor.matmul(ps, sb['W10l1'], h9s[1], start=False,
                                 stop=True)
                h10s = stpool.tile([128, 512], f32, tag="h10", name="h10s")
                nc.scalar.activation(h10s, ps, AF.Prelu, bias=sb['b10c'],
                                     alpha=NEG)
                ps2 = mmp.tile([128, 512], f32, tag="mm")
                nc.tensor.matmul(ps2[0:50, :], sb['W11l'], h10s, start=True,
                                 stop=True)
                osb = stpool.tile([50, 512], f32, tag="osb", name="osb")
                nc.scalar.copy(osb[:], ps2[0:50, :])
                nc.sync.dma_start(out_d[:, cs], osb[:])

    return nc


def _get_program(const_shapes):
    if "prog" not in _CACHE:
        nc = _build_program(const_shapes)
        _split_excess_waits(nc)
        _CACHE["prog"] = nc
    return _CACHE["prog"]


def kernel(x, params):
    from concourse.bass_utils import run_bass_kernel_spmd

    x = np.asarray(x, np.float32)
    consts = _prep_consts(params)
    const_shapes = {k: v.shape for k, v in consts.items()}
    nc = _get_program(const_shapes)

    in_maps = []
    for b in range(B):
        x_dl, x_dr = _prep_x(x[b])
        m = dict(consts)
        m['x_dl'] = x_dl
        m['x_dr'] = x_dr
        in_maps.append(m)

    res = run_bass_kernel_spmd(nc, in_maps, core_ids=list(range(B)))
    return np.stack([res.results[b]['out'] for b in range(B)], 0)
